# revision 11
# baseline (speedup 1.0000x reference)
"""ConcatCritic pair-scorer on 8 TRN2 cores.

reference:  out[a, c] = W2 . relu(concat(x[a], y[c]) @ W1 + b1) + b2
factorized: out[a, c] = W2 . relu(Xp[a, :] + Yp[c, :] + b1) + b2
            with Xp = x @ W1[:D],  Yp = y @ W1[D:]

Sharding: data-parallel over the x/batch rows (64 rows per core). Each core
holds full y, W1, b1, W2 and computes its [64, 512] stripe of the output.
Inputs are fed pre-transposed from the host (xT, yT, b1T, w2T) so the device
does no layout shuffling.

Per-core (h = hidden index on partitions, 4 h-tiles of 128):
  Y_kt [128h, 512c] = (y @ W1y)^T tile      via matmul(lhsT=W1y[:,hs], rhs=yT)
  q_kt [128h,  64a] = (x @ W1x)^T + b1      via matmul(lhsT=W1x[:,hs], rhs=xT)
  main loop over a (64) x kt (4):
    u = relu(Y_kt + q_kt[:, a])        on DVE (tensor_scalar add+max) or ACT
    psum[1, 512c] += w2T[:, kt].T @ u  on PE (M=1 matmul, accumulate over kt)
  copy psum row -> SBUF, DMA -> out[a, :]
"""

from contextlib import ExitStack

import numpy as np

import concourse.bass as bass
import concourse.bacc as bacc
import concourse.mybir as mybir
import concourse.tile as tile
from concourse.bass_utils import run_bass_kernel_spmd

B = 512
D = 128
H = 512
NCORES = 8
BS = B // NCORES  # 64 x-rows per core
KT = H // 128  # 4 h-tiles
FP = mybir.dt.float32

# u-tile dtype (tuned empirically)
U_DT = mybir.dt.float32


def _relu_engine(a: int, kt: int) -> str:
    """Producer engine per (a, kt) tile: 'v' = VectorE, 's' = ScalarE.

    kt == 0 must be ScalarE: the first matmul of each psum accumulation
    group also waits on the psum-slot release (the ScalarE score copy), and
    PE matmuls support only ONE sync wait — producer and release must be
    the same semaphore.
    """
    return "s" if kt == 0 else "v"

_NC = None
LAST_RESULTS = None


def _build_nc():
    nc = bacc.Bacc(None, target_bir_lowering=False, num_devices=NCORES)
    xT = nc.dram_tensor("xT", [D, BS], FP, kind="ExternalInput")
    yT = nc.dram_tensor("yT", [D, B], FP, kind="ExternalInput")
    W1 = nc.dram_tensor("W1", [2 * D, H], FP, kind="ExternalInput")
    b1T = nc.dram_tensor("b1T", [128, KT], FP, kind="ExternalInput")
    w2T = nc.dram_tensor("w2T", [128, KT], FP, kind="ExternalInput")
    out = nc.dram_tensor("out", [BS, B], FP, kind="ExternalOutput")

    with tile.TileContext(nc) as tc, ExitStack() as ctx:
        const = ctx.enter_context(tc.tile_pool(name="const", bufs=1))
        setup_ps = ctx.enter_context(tc.tile_pool(name="setup_ps", bufs=2, space="PSUM"))
        score_ps = ctx.enter_context(tc.tile_pool(name="score_ps", bufs=4, space="PSUM"))
        score_sb = ctx.enter_context(tc.tile_pool(name="score_sb", bufs=4))
        upool = ctx.enter_context(tc.tile_pool(name="u", bufs=6))

        W1x_sb = const.tile([128, H], FP, tag="W1x")
        nc.sync.dma_start(W1x_sb[:], W1[0:D, :])
        W1y_sb = const.tile([128, H], FP, tag="W1y")
        nc.sync.dma_start(W1y_sb[:], W1[D : 2 * D, :])
        xT_sb = const.tile([128, BS], FP, tag="xT")
        nc.sync.dma_start(xT_sb[:], xT[:, :])
        yT_sb = const.tile([128, B], FP, tag="yT")
        nc.sync.dma_start(yT_sb[:], yT[:, :])
        b1T_sb = const.tile([128, KT], FP, tag="b1T")
        nc.sync.dma_start(b1T_sb[:], b1T[:, :])
        w2T_sb = const.tile([128, KT], FP, tag="w2T")
        nc.sync.dma_start(w2T_sb[:], w2T[:, :])

        # PE matmuls support only ONE sync wait. Warm the PE's view of every
        # input-DMA semaphore with tiny self-referencing matmuls (one new
        # semaphore each) so no real matmul ever needs two waits.
        warm_ps = ctx.enter_context(tc.tile_pool(name="warm_ps", bufs=1, space="PSUM"))
        warm = warm_ps.tile([128, KT], FP, tag="warm")
        warm_mms = []
        for src in (w2T_sb, W1y_sb, W1x_sb, yT_sb, xT_sb):
            mm = nc.tensor.matmul(
                warm[:1, :KT], src[:, 0:1], src[:, 0:KT], start=True, stop=True
            )
            warm_mms.append(mm)

        # Y_kt = (y @ W1y)^T tiles, q_kt = (x @ W1x)^T + b1
        Y = []
        Q = []
        first_mm = None
        for kt in range(KT):
            hs = slice(kt * 128, (kt + 1) * 128)
            ps_Y = setup_ps.tile([128, B], FP, tag="setup")
            mm_Y = nc.tensor.matmul(ps_Y[:], W1y_sb[:, hs], yT_sb[:], start=True, stop=True)
            if first_mm is None:
                first_mm = mm_Y
                for wmm in warm_mms:
                    tile.add_dep_helper(
                        first_mm.ins, wmm.ins, sync=True, reason="PE 1-wait warmup"
                    )
            Ykt = const.tile([128, B], U_DT, tag=f"Y{kt}")
            nc.vector.tensor_copy(Ykt[:], ps_Y[:])
            Y.append(Ykt)

            ps_q = setup_ps.tile([128, BS], FP, tag="setup")
            nc.tensor.matmul(ps_q[:], W1x_sb[:, hs], xT_sb[:], start=True, stop=True)
            qkt = const.tile([128, BS], FP, tag=f"q{kt}")
            nc.vector.tensor_scalar(
                qkt[:], ps_q[:], b1T_sb[:, kt : kt + 1], None, mybir.AluOpType.add
            )
            Q.append(qkt)

        # main loop: 64 output rows, each = 4 accumulating M=1 matmuls
        for a in range(BS):
            ps_s = score_ps.tile([1, B], FP, tag="ps_s")
            for kt in range(KT):
                u = upool.tile([128, B], U_DT, tag="u")
                bias_col = Q[kt][:, a : a + 1]
                if _relu_engine(a, kt) == "v":
                    nc.vector.tensor_scalar(
                        u[:],
                        Y[kt][:],
                        bias_col,
                        0.0,
                        mybir.AluOpType.add,
                        mybir.AluOpType.max,
                    )
                else:
                    nc.scalar.activation(
                        u[:],
                        Y[kt][:],
                        mybir.ActivationFunctionType.Relu,
                        bias=bias_col,
                    )
                nc.tensor.matmul(
                    ps_s[:],
                    w2T_sb[:, kt : kt + 1],
                    u[:],
                    start=(kt == 0),
                    stop=(kt == KT - 1),
                )
            sb_s = score_sb.tile([1, B], FP, tag="sb_s")
            nc.scalar.copy(sb_s[:], ps_s[:])
            nc.sync.dma_start(out[a : a + 1, :], sb_s[:])

    nc.finalize()
    return nc


def kernel(**inputs) -> np.ndarray:
    global _NC, LAST_RESULTS
    if _NC is None:
        _NC = _build_nc()

    x = np.asarray(inputs["x"], dtype=np.float32)
    y = np.asarray(inputs["y"], dtype=np.float32)
    W1 = np.ascontiguousarray(inputs["W1"], dtype=np.float32)
    b1 = np.asarray(inputs["b1"], dtype=np.float32)
    W2 = np.asarray(inputs["W2"], dtype=np.float32)
    b2 = np.asarray(inputs["b2"], dtype=np.float32)

    yT = np.ascontiguousarray(y.T)
    b1T = np.ascontiguousarray(b1.reshape(KT, 128).T)
    w2T = np.ascontiguousarray(W2[:, 0].reshape(KT, 128).T)

    in_maps = [
        {
            "xT": np.ascontiguousarray(x[m * BS : (m + 1) * BS].T),
            "yT": yT,
            "W1": W1,
            "b1T": b1T,
            "w2T": w2T,
        }
        for m in range(NCORES)
    ]
    LAST_RESULTS = run_bass_kernel_spmd(_NC, in_maps, list(range(NCORES)))
    S = np.concatenate([LAST_RESULTS.results[m]["out"] for m in range(NCORES)], axis=0)
    return (S + b2[0]).astype(np.float32)


# revision 13
# speedup vs baseline: 2.5866x; 2.5866x over previous
"""ConcatCritic pair-scorer on 8 TRN2 cores.

reference:  out[a, c] = W2 . relu(concat(x[a], y[c]) @ W1 + b1) + b2
factorized: out[a, c] = W2 . relu(Xp[a, :] + Yp[c, :] + b1) + b2
            with Xp = x @ W1[:D],  Yp = y @ W1[D:]

Sharding: data-parallel over the x/batch rows (64 rows per core). Each core
holds full y, W1, b1, W2 and computes its [64, 512] stripe of the output.
Inputs are fed pre-transposed from the host (xT, yT, b1T, w2T) so the device
does no layout shuffling.

Per-core dataflow (h = hidden index on partitions, 4 h-tiles of 128):
  setup (fp32 on PE):
    Y_kt [128h, 512c] = (y @ W1y)^T tile   via matmul(lhsT=W1y[:,hs], rhs=yT)
    q_kt [128h,  64a] = (x @ W1x)^T + b1   via matmul(lhsT=W1x[:,hs], rhs=xT)
    both rounded to bf16 SBUF tiles.
  main loop (64 output rows in 16 groups of 4, 2 blocks of 8 groups):
    u[s] = relu(Y_kt + q_kt[:, a])  bf16, on DVE (tensor_scalar add+max,
                                    4x mode) or ACT (activation Relu + bias)
    psum_g[32j, 512c] += w2T[:, kt].T @ u   M=1 bf16 matmul; the 4 rows of a
                                    group go to partitions {0,32,64,96} of one
                                    PSUM bank via tile_position=(0, 32j), so
                                    4 matmuls stream concurrently on separate
                                    PE column-groups.
    per group: one ACT/DVE copy PSUM->SBUF (f32), 4 row-DMAs to out.

All tiles are fixed allocations (no pool slot rotation) so cross-engine
slot-release waits never stack up: TPB instructions carry at most ONE sync
wait; Bacc's generate_event_semaphores legalizes any extras into
EventSemaphore instructions, which serialize the engine queues - the layout
below keeps those rare.
"""

from contextlib import ExitStack

import ml_dtypes
import numpy as np

import concourse.bacc as bacc
import concourse.bass as bass
import concourse.mybir as mybir
import concourse.tile as tile
from concourse.bass_utils import run_bass_kernel_spmd

B = 512
D = 128
H = 512
NCORES = 8
BS = B // NCORES  # 64 x-rows per core
KT = H // 128  # 4 h-tiles
GROUPS = BS // 4  # 16 groups of 4 output rows
GPB = 8  # groups per block (8 PSUM banks)
NBLK = GROUPS // GPB
FP = mybir.dt.float32
BF = mybir.dt.bfloat16

_NC = None
LAST_RESULTS = None


def _act_relu(kt: int, g: int, j: int) -> bool:
    """Which relu tiles ScalarE produces (rest go to VectorE).

    (kt==0, j==0) tiles MUST be ScalarE: the first matmul touching a psum
    bank also carries the bank's WAR release (the ScalarE score copy of the
    previous block), and PE matmuls support only one sync wait - producer
    and release must be the same semaphore.  The extra ACT tiles balance
    engine time (ACT ~400ns vs DVE ~194ns per tile).
    """
    if j == 0:
        return True  # 8 per (kt) phase: covers the kt==0 constraint
    return j == 1 and g < 3  # ~3 more per phase for balance


def _copy_engine(gg: int) -> str:
    # split the 16 PSUM->SBUF score copies between ACT and DVE
    return "s" if gg % 2 == 0 else "v"


def _build_nc():
    nc = bacc.Bacc(None, target_bir_lowering=False, num_devices=NCORES)
    xT = nc.dram_tensor("xT", [D, BS], FP, kind="ExternalInput")
    yT = nc.dram_tensor("yT", [D, B], FP, kind="ExternalInput")
    W1 = nc.dram_tensor("W1", [2 * D, H], FP, kind="ExternalInput")
    b1T = nc.dram_tensor("b1T", [128, KT], FP, kind="ExternalInput")
    w2T = nc.dram_tensor("w2T", [128, KT], BF, kind="ExternalInput")
    out = nc.dram_tensor("out", [BS, B], FP, kind="ExternalOutput")

    with tile.TileContext(nc) as tc, ExitStack() as ctx:
        const = ctx.enter_context(tc.tile_pool(name="const", bufs=1))

        W1x_sb = const.tile([128, H], FP, tag="W1x")
        nc.sync.dma_start(W1x_sb[:], W1[0:D, :])
        W1y_sb = const.tile([128, H], FP, tag="W1y")
        nc.sync.dma_start(W1y_sb[:], W1[D : 2 * D, :])
        xT_sb = const.tile([128, BS], FP, tag="xT")
        nc.sync.dma_start(xT_sb[:], xT[:, :])
        yT_sb = const.tile([128, B], FP, tag="yT")
        nc.sync.dma_start(yT_sb[:], yT[:, :])
        b1T_sb = const.tile([128, KT], FP, tag="b1T")
        nc.sync.dma_start(b1T_sb[:], b1T[:, :])
        w2T_sb = const.tile([128, KT], BF, tag="w2T")
        nc.sync.dma_start(w2T_sb[:], w2T[:, :])

        # warm DVE's view of b1T's DMA semaphore (keeps the q_kt tensor_scalar
        # at one wait)
        scratch = const.tile([128, KT], FP, tag="scratch")
        nc.vector.tensor_copy(scratch[:], b1T_sb[:])

        # fixed PSUM tiles: 8 score banks, reused across the 2 blocks
        score_ps = ctx.enter_context(tc.tile_pool(name="score_ps", bufs=1, space="PSUM"))
        ps = [score_ps.tile([128, B], FP, tag=f"ps{g}", name=f"ps{g}") for g in range(GPB)]
        # setup matmuls ping-pong between two dedicated psum tiles; they are
        # only live during setup, then stay unused (SBUF->PSUM budget: 8+0,
        # setup reuses score banks ps[0], ps[1] AFTER warmup? no - separate:
        # warm + setup writes land in ps banks before any score matmul and
        # are WAW-overwritten by the first start=True score matmul (same
        # engine, no extra waits).

        # PE matmuls support only ONE sync wait.  Warm the PE's view of every
        # input-DMA semaphore with tiny self-referencing matmuls (one new
        # semaphore each) so no real matmul ever needs two waits.
        warm_mms = []
        for src in (w2T_sb, W1y_sb, W1x_sb, yT_sb, xT_sb):
            mm = nc.tensor.matmul(
                ps[0][:1, :KT], src[:, 0:1], src[:, 0:KT], start=True, stop=True
            )
            warm_mms.append(mm)

        # Y_kt = (y @ W1y)^T tiles, q_kt = (x @ W1x)^T + b1 (bf16 for the
        # 4x-mode DVE relu; q stays f32 - it is only read as a per-partition
        # scalar)
        Y = []
        Q = []
        first_mm = None
        for kt in range(KT):
            hs = slice(kt * 128, (kt + 1) * 128)
            ps_Y = ps[2 * (kt % 2)]
            mm_Y = nc.tensor.matmul(ps_Y[:], W1y_sb[:, hs], yT_sb[:], start=True, stop=True)
            if first_mm is None:
                first_mm = mm_Y
                for wmm in warm_mms:
                    tile.add_dep_helper(
                        first_mm.ins, wmm.ins, sync=True, reason="PE 1-wait warmup"
                    )
            Ykt = const.tile([128, B], BF, tag=f"Y{kt}")
            nc.vector.tensor_copy(Ykt[:], ps_Y[:])
            Y.append(Ykt)

            ps_q = ps[2 * (kt % 2) + 1]
            nc.tensor.matmul(ps_q[:, :BS], W1x_sb[:, hs], xT_sb[:], start=True, stop=True)
            qkt = const.tile([128, BS], FP, tag=f"q{kt}")
            nc.vector.tensor_scalar(
                qkt[:], ps_q[:, :BS], b1T_sb[:, kt : kt + 1], None, mybir.AluOpType.add
            )
            Q.append(qkt)

        # fixed u tiles (one per (g, j) position in a phase) and output
        # staging tiles (one per group, never reused)
        U = [const.tile([128, B], BF, tag=f"u{s}", name=f"u{s}") for s in range(4 * GPB)]
        SB = [const.tile([128, B], FP, tag=f"o{gg}", name=f"o{gg}") for gg in range(GROUPS)]

        for blk in range(NBLK):
            for kt in range(KT):
                for g in range(GPB):
                    gg = blk * GPB + g
                    for j in range(4):
                        a = gg * 4 + j
                        s = g * 4 + j
                        u = U[s]
                        bias_col = Q[kt][:, a : a + 1]
                        if _act_relu(kt, g, j):
                            nc.scalar.activation(
                                u[:],
                                Y[kt][:],
                                mybir.ActivationFunctionType.Relu,
                                bias=bias_col,
                            )
                        else:
                            nc.vector.tensor_scalar(
                                u[:],
                                Y[kt][:],
                                bias_col,
                                0.0,
                                mybir.AluOpType.add,
                                mybir.AluOpType.max,
                            )
                        nc.tensor.matmul(
                            ps[g][32 * j : 32 * j + 1, :],
                            w2T_sb[:, kt : kt + 1],
                            u[:],
                            start=(kt == 0),
                            stop=(kt == KT - 1),
                            tile_position=(0, 32 * j),
                            skip_group_check=True,
                        )
            for g in range(GPB):
                gg = blk * GPB + g
                sb = SB[gg]
                if _copy_engine(gg) == "s":
                    nc.scalar.copy(sb[:], ps[g][:])
                else:
                    nc.vector.tensor_copy(sb[:], ps[g][:])
                for j in range(4):
                    nc.sync.dma_start(
                        out[gg * 4 + j : gg * 4 + j + 1, :],
                        sb[32 * j : 32 * j + 1, :],
                    )

    nc.finalize()
    return nc


def kernel(**inputs) -> np.ndarray:
    global _NC, LAST_RESULTS
    if _NC is None:
        _NC = _build_nc()

    x = np.asarray(inputs["x"], dtype=np.float32)
    y = np.asarray(inputs["y"], dtype=np.float32)
    W1 = np.ascontiguousarray(inputs["W1"], dtype=np.float32)
    b1 = np.asarray(inputs["b1"], dtype=np.float32)
    W2 = np.asarray(inputs["W2"], dtype=np.float32)
    b2 = np.asarray(inputs["b2"], dtype=np.float32)

    yT = np.ascontiguousarray(y.T)
    b1T = np.ascontiguousarray(b1.reshape(KT, 128).T)
    w2T = np.ascontiguousarray(W2[:, 0].reshape(KT, 128).T.astype(ml_dtypes.bfloat16))

    in_maps = [
        {
            "xT": np.ascontiguousarray(x[m * BS : (m + 1) * BS].T),
            "yT": yT,
            "W1": W1,
            "b1T": b1T,
            "w2T": w2T,
        }
        for m in range(NCORES)
    ]
    LAST_RESULTS = run_bass_kernel_spmd(_NC, in_maps, list(range(NCORES)))
    S = np.concatenate([LAST_RESULTS.results[m]["out"] for m in range(NCORES)], axis=0)
    return (S + b2[0]).astype(np.float32)
